# revision 15
# baseline (speedup 1.0000x reference)
"""3x3 median filter (reflect padding) on Trainium2, data-parallel over batch.

Input:  image [16, 3, 512, 512] f32
Output: same shape; out[b,c,y,x] = median of the 3x3 window around (y,x),
        reflect padding.

Sharding: batch dim split across 8 NeuronCores (2 images per core), SPMD.

All VectorE TENSOR_TENSOR ops run in bf16 with dense step-1, 4-byte-aligned
access patterns so every one hits the DVE 2x_1P perf mode (2 elem/cycle/lane)
instead of the 1x floor fp32 TT is stuck at. bf16 keeps rel-err ~2^-8
(<< 2e-2 tolerance) with a full 8-bit exponent (no subnormal blowup near
the harness' 1e-6 denom floor).

Host prep: per-core input is transposed+padded to [BPC, H+2, C, W+2] bf16
with BOTH the vertical and horizontal reflect borders pre-staged. The
horizontal pad removes all boundary-column special cases; every op is a
uniform dense sweep over flattened [C, W+2] (the 2 pad cols per channel
compute garbage that is simply not stored).

Layout: 2-row strips. Each megastep covers a 256-row half-image; partition
p owns output rows 2p and 2p+1 and loads its 4 window rows (2p..2p+3
padded) with ONE contiguous DMA (1.5x HBM read amplification vs 3x for
row-per-partition layouts). The two output rows share their middle
vertical pair, cutting the vertical sort from 6 to 5 min/max ops per row:
  PM = min/max(row1, row2)                     (2 TT, shared by both rows)
  per row: m, hi, md, lo from PM + third row   (8 TT)
Horizontal merge per megastep (ScalarE makes the single odd-shift copy
s1[k][x]=lmh[k][x+1]; everything else is even offsets):
  pairs:   Pmax_lo,Pmax_md / Pmin_md,Pmin_hi   (2 stacked TT)
  combine: t3, Cc (stacked) / A / B            (3 TT)
  median = med3(A, B, Cc)                      (4 TT)
Software pipeline: stage_a(t+1) [DMA+vertical+s1] is emitted before
stage_b(t) [pairs..median] so the ScalarE shift-copy never stalls VectorE.
The first megastep's DMA is split so compute starts after the 2 middle
rows land; the last megastep's median+store are split by row parity so
the final DMA drains only half the result.
"""

import sys

sys.path.insert(0, "/opt/trn_rl_repo")

import numpy as np

_COMPILED = {}

B, C, H, W = 16, 3, 512, 512
NCORES = 8
BPC = B // NCORES  # batches per core
RT = 128           # partitions
NMS = 2            # megasteps (half-images) per batch
MH = H // NMS      # rows per megastep (256)
HP = H + 2         # padded rows on device
WP = W + 2         # padded cols on device
SRP = C * WP       # padded row stride (elements), 1542
SB = HP * SRP      # batch stride (input)
SR = C * W         # output row stride
SBO = H * SR       # batch stride (output)
FDS = SRP          # one row-slice, flattened [C, WP]
FDC = 2 * WP + W   # combine width 1540: covers flat c*WP+x, x<W
LMT = 3 * FDS      # lmh row stride
ST = 4 * FDS       # S row stride
TCT = 2 * FDC      # TC row stride
NS1 = 2 * LMT      # full s1 flat span (9252)


def _legalize_waits(nc, mybir):
    """Hoist excess sync-waits into a preceding same-engine EventSemaphore.
    The TRN2 ISA allows 1 sync-wait on compute instructions (EventSemaphore
    allows more) but Tile's scheduler can emit more; a wait-only instruction
    earlier in the same engine's program order is semantically identical."""
    limits = {"InstEventSemaphore": 2}
    n_hoisted = 0
    for f in nc.m.functions:
        for bb in f.blocks:
            il = bb.instructions
            idx = 0
            while idx < len(il):
                i = il[idx]
                si = i.sync_info
                lim = limits.get(type(i).__name__, 1)
                if si is not None and si.on_wait and len(si.on_wait) > lim:
                    waits = list(si.on_wait)
                    keep, excess = waits[:lim], waits[lim:]
                    hoists = []
                    for j in range(0, len(excess), 2):
                        h = mybir.InstEventSemaphore(
                            name=f"hoistw_{n_hoisted}", ins=[], outs=[])
                        n_hoisted += 1
                        h.engine = i.engine
                        h.sync_info = mybir.SyncInfo(
                            on_wait=excess[j:j + 2], on_update=[])
                        hoists.append(h)
                    i.sync_info = mybir.SyncInfo(
                        on_wait=keep, on_update=si.on_update)
                    for k, h in enumerate(hoists):
                        il.insert(idx + k, h)
                    idx += len(hoists)
                idx += 1
    return n_hoisted


def _build_nc():
    from concourse import bass
    import concourse.mybir as mybir
    from concourse.tile import TileContext

    bf16 = mybir.dt.bfloat16
    MIN = mybir.AluOpType.min
    MAX = mybir.AluOpType.max
    AP = bass.AP

    nc = bass.Bass()
    img = nc.dram_tensor("image", [BPC, HP, C, WP], bf16, kind="ExternalInput")
    out = nc.dram_tensor("out", [BPC, H, C, W], bf16, kind="ExternalOutput")

    def fv(tile_ap, off, dims):
        """Free-dim view of an SBUF tile: keep partition dim, replace free
        dims with `dims`, shift base by `off` elements."""
        return AP(tile_ap.tensor, tile_ap.offset + off,
                  [list(tile_ap.ap[0])] + [list(d) for d in dims])

    def stage_a(nc, pool, g, h, split_dma):
        """DMA 4 window rows/partition + shared-pair vertical sort3 +
        ScalarE shift copy. Partition p owns output rows MH*h+2p, +2p+1;
        in4[k] = padded row MH*h + 2p + k.

        Returns (lmh, s1), both [RT, 2(row), 3(slice), C, WP].
        """
        base = g * SB + h * MH * SRP
        in4 = pool.tile([RT, 4, C, WP], bf16, tag="in4")
        i4 = in4[:]
        if split_dma:
            # middle rows first: the shared pair needs only in4[1..2]
            nc.sync.dma_start(
                out=fv(i4, FDS, [[1, 2 * FDS]]),
                in_=AP(img, base + SRP, [[2 * SRP, RT], [1, 2 * FDS]]))
            nc.sync.dma_start(
                out=fv(i4, 0, [[1, FDS]]),
                in_=AP(img, base, [[2 * SRP, RT], [1, FDS]]))
            nc.sync.dma_start(
                out=fv(i4, 3 * FDS, [[1, FDS]]),
                in_=AP(img, base + 3 * SRP, [[2 * SRP, RT], [1, FDS]]))
        else:
            nc.sync.dma_start(
                out=fv(i4, 0, [[1, 4 * FDS]]),
                in_=AP(img, base, [[2 * SRP, RT], [1, 4 * FDS]]))

        TT = nc.vector.tensor_tensor
        # shared vertical pair of the strip: PM[0]=min, PM[1]=max of rows 1,2
        PM = pool.tile([RT, 2, C, WP], bf16, tag="PM", bufs=1)
        pmn = fv(PM[:], 0, [[1, FDS]])
        pmx = fv(PM[:], FDS, [[1, FDS]])
        TT(pmn, fv(i4, FDS, [[1, FDS]]), fv(i4, 2 * FDS, [[1, FDS]]), MIN)
        TT(pmx, fv(i4, FDS, [[1, FDS]]), fv(i4, 2 * FDS, [[1, FDS]]), MAX)

        # combine with the third row (in4[0] for row0, in4[3] for row1),
        # both rows stacked per instruction; the shared PM slices enter via
        # 0-stride broadcast dims:
        # m=min(pmx,3rd) hi=max(pmx,3rd) md=max(pmn,m) lo=min(pmn,m)
        lmh = pool.tile([RT, 2, 3, C, WP], bf16, tag="lmh")
        m = pool.tile([RT, 2, C, WP], bf16, tag="m", bufs=1)
        lv = lmh[:]
        thirds = fv(i4, 0, [[3 * FDS, 2], [1, FDS]])
        pmn_b = fv(PM[:], 0, [[0, 2], [1, FDS]])
        pmx_b = fv(PM[:], FDS, [[0, 2], [1, FDS]])
        mm = fv(m[:], 0, [[FDS, 2], [1, FDS]])
        lo = fv(lv, 0 * FDS, [[LMT, 2], [1, FDS]])
        md = fv(lv, 1 * FDS, [[LMT, 2], [1, FDS]])
        hi = fv(lv, 2 * FDS, [[LMT, 2], [1, FDS]])
        TT(mm, pmx_b, thirds, MIN)
        TT(hi, pmx_b, thirds, MAX)
        TT(md, pmn_b, mm, MAX)
        TT(lo, pmn_b, mm, MIN)

        # the single odd shift, on ScalarE: s1[r][k][x] = lmh[r][k][x+1]
        s1 = pool.tile([RT, 2, 3, C, WP], bf16, tag="s1")
        nc.scalar.copy(fv(s1[:], 0, [[1, NS1 - 1]]),
                       fv(lv, 1, [[1, NS1 - 1]]))
        # last element feeds only a discarded pad column; init to keep
        # uninitialized-read checks quiet (GpSimd so VectorE loses no slot)
        nc.gpsimd.memset(fv(s1[:], NS1 - 1, [[1, 1]]), 0.0)
        return lmh, s1

    def stage_b(nc, pool, g, h, lmh, s1, split_tail):
        """Pairs + combine + med3 + DMA out, consuming stage_a tiles."""
        lv, sv = lmh[:], s1[:]
        TT = nc.vector.tensor_tensor
        # S: 0=Pmax_lo 1=Pmax_md 2=Pmin_md 3=Pmin_hi (per row)
        S = pool.tile([RT, 2, 4, C, WP], bf16, tag="S", bufs=1)
        Sv = S[:]
        d2 = [[LMT, 2], [FDS, 2], [1, FDS]]
        TT(fv(Sv, 0, [[ST, 2], [FDS, 2], [1, FDS]]),
           fv(lv, 0, d2), fv(sv, 0, d2), MAX)
        TT(fv(Sv, 2 * FDS, [[ST, 2], [FDS, 2], [1, FDS]]),
           fv(lv, FDS, d2), fv(sv, FDS, d2), MIN)

        # combine with even +2 shifts:
        # TC[r][0]=t3=min(Pmax_md, md+2); TC[r][1]=Cc=min(Pmin_hi, hi+2)
        TC = pool.tile([RT, 2, 2, FDC], bf16, tag="TC", bufs=1)
        A = pool.tile([RT, 2, FDC], bf16, tag="A", bufs=1)
        Bt = pool.tile([RT, 2, FDC], bf16, tag="Bt", bufs=1)
        TT(fv(TC[:], 0, [[TCT, 2], [FDC, 2], [1, FDC]]),
           fv(Sv, FDS, [[ST, 2], [2 * FDS, 2], [1, FDC]]),
           fv(lv, FDS + 2, [[LMT, 2], [FDS, 2], [1, FDC]]),
           MIN)
        TT(fv(A[:], 0, [[FDC, 2], [1, FDC]]),
           fv(Sv, 0, [[ST, 2], [1, FDC]]),
           fv(lv, 2, [[LMT, 2], [1, FDC]]),
           MAX)
        TT(fv(Bt[:], 0, [[FDC, 2], [1, FDC]]),
           fv(Sv, 2 * FDS, [[ST, 2], [1, FDC]]),
           fv(TC[:], 0, [[TCT, 2], [1, FDC]]),
           MAX)

        # final med3(A, B, Cc) (A in-place), then store per row parity:
        # out row MH*h + 2p + r lives at res[r]; out[.., c, x] = res[r][c*WP+x]
        f1 = pool.tile([RT, 2, FDC], bf16, tag="f1", bufs=1)
        res = pool.tile([RT, 2, FDC], bf16, tag="res")
        parities = ((0, 1), (1, 1)) if split_tail else ((0, 2),)
        for r0, nr in parities:
            dc = [[FDC, nr], [1, FDC]]
            off = r0 * FDC
            Av = fv(A[:], off, dc)
            Bv = fv(Bt[:], off, dc)
            Fv = fv(f1[:], off, dc)
            Cv = fv(TC[:], r0 * TCT + FDC, [[TCT, nr], [1, FDC]])
            TT(Fv, Av, Bv, MIN)
            TT(Av, Av, Bv, MAX)
            TT(Av, Av, Cv, MIN)
            TT(fv(res[:], off, dc), Fv, Av, MAX)
            for r in range(r0, r0 + nr):
                nc.sync.dma_start(
                    out=AP(out, g * SBO + (h * MH + r) * SR,
                           [[2 * SR, RT], [W, C], [1, W]]),
                    in_=fv(res[:], r * FDC, [[WP, C], [1, W]]))

    with TileContext(nc) as tc:
        with tc.tile_pool(name="p", bufs=2) as pool:
            # Software pipeline: emit stage_a(t+1) before stage_b(t) so the
            # ScalarE shift-copy of megastep t overlaps VectorE's vertical
            # sort of megastep t+1 instead of stalling the pair stage.
            steps = [(g, h) for g in range(BPC) for h in range(NMS)]
            prev = None
            for i, (g, h) in enumerate(steps):
                cur = stage_a(nc, pool, g, h, split_dma=(i == 0))
                if prev is not None:
                    stage_b(nc, pool, *prev[0], *prev[1], split_tail=False)
                prev = ((g, h), cur)
            stage_b(nc, pool, *prev[0], *prev[1], split_tail=True)

    _legalize_waits(nc, mybir)
    return nc


def _stage_input(img_k: np.ndarray) -> np.ndarray:
    """[BPC, C, H, W] f32 -> reflect-padded transposed [BPC, HP, C, WP] bf16."""
    import ml_dtypes

    t = img_k.transpose(0, 2, 1, 3).astype(ml_dtypes.bfloat16)  # [BPC,H,C,W]
    p = np.empty((BPC, HP, C, WP), dtype=ml_dtypes.bfloat16)
    p[:, 1:H + 1, :, 1:W + 1] = t
    p[:, 1:H + 1, :, 0] = t[:, :, :, 1]          # col -1 = col 1
    p[:, 1:H + 1, :, W + 1] = t[:, :, :, W - 2]  # col W  = col W-2
    p[:, 0] = p[:, 2]          # row -1 = row 1
    p[:, H + 1] = p[:, H - 1]  # row H  = row H-2
    return p


def kernel(image: np.ndarray) -> np.ndarray:
    from concourse.bass_utils import run_bass_kernel_spmd

    image = np.asarray(image, dtype=np.float32)
    if "nc" not in _COMPILED:
        _COMPILED["nc"] = _build_nc()
    nc = _COMPILED["nc"]

    in_maps = [{"image": _stage_input(image[k * BPC:(k + 1) * BPC])}
               for k in range(NCORES)]
    try:
        res = run_bass_kernel_spmd(nc, in_maps, core_ids=list(range(NCORES)))
    except Exception:
        # transient accelerator errors (e.g. NRT_EXEC_UNIT_UNRECOVERABLE)
        # have been observed to clear on retry
        res = run_bass_kernel_spmd(nc, in_maps, core_ids=list(range(NCORES)))
    return np.concatenate(
        [np.asarray(res.results[k]["out"]).astype(np.float32)
         .transpose(0, 2, 1, 3) for k in range(NCORES)],
        axis=0)


# revision 16
# speedup vs baseline: 1.0066x; 1.0066x over previous
"""3x3 median filter (reflect padding) on Trainium2, data-parallel over batch.

Input:  image [16, 3, 512, 512] f32
Output: same shape; out[b,c,y,x] = median of the 3x3 window around (y,x),
        reflect padding.

Sharding: batch dim split across 8 NeuronCores (2 images per core), SPMD.

All VectorE TENSOR_TENSOR ops run in bf16 with dense step-1, 4-byte-aligned
access patterns so every one hits the DVE 2x_1P perf mode (2 elem/cycle/lane)
instead of the 1x floor fp32 TT is stuck at. bf16 keeps rel-err ~2^-8
(<< 2e-2 tolerance) with a full 8-bit exponent (no subnormal blowup near
the harness' 1e-6 denom floor).

Host prep: per-core input is transposed+padded to [BPC, H+2, C, W+2] bf16
with BOTH the vertical and horizontal reflect borders pre-staged. The
horizontal pad removes all boundary-column special cases; every op is a
uniform dense sweep over flattened [C, W+2] (the 2 pad cols per channel
compute garbage that is simply not stored).

Layout: 2-row strips. Each megastep covers a 256-row half-image; partition
p owns output rows 2p and 2p+1 and loads its 4 window rows (2p..2p+3
padded) with ONE contiguous DMA (1.5x HBM read amplification vs 3x for
row-per-partition layouts). The two output rows share their middle
vertical pair, cutting the vertical sort from 6 to 5 min/max ops per row:
  PM = min/max(row1, row2)                     (2 TT, shared by both rows)
  per row: m, hi, md, lo from PM + third row   (8 TT)
Horizontal merge per megastep (ScalarE makes the single odd-shift copy
s1[k][x]=lmh[k][x+1]; everything else is even offsets):
  pairs:   Pmax_lo,Pmax_md / Pmin_md,Pmin_hi   (2 stacked TT)
  combine: t3, Cc (stacked) / A / B            (3 TT)
  median = med3(A, B, Cc)                      (4 TT)
Software pipeline: stage_a(t+1) [DMA+vertical+s1] is emitted before
stage_b(t) [pairs..median] so the ScalarE shift-copy never stalls VectorE.
The first megastep's DMA is split so compute starts after the 2 middle
rows land; the last megastep's median+store are split by row parity so
the final DMA drains only half the result.
"""

import sys

sys.path.insert(0, "/opt/trn_rl_repo")

import numpy as np

_COMPILED = {}

B, C, H, W = 16, 3, 512, 512
NCORES = 8
BPC = B // NCORES  # batches per core
RT = 128           # partitions
NMS = 2            # megasteps (half-images) per batch
MH = H // NMS      # rows per megastep (256)
HP = H + 2         # padded rows on device
WP = W + 2         # padded cols on device
SRP = C * WP       # padded row stride (elements), 1542
SB = HP * SRP      # batch stride (input)
SR = C * W         # output row stride
SBO = H * SR       # batch stride (output)
FDS = SRP          # one row-slice, flattened [C, WP]
FDC = 2 * WP + W   # combine width 1540: covers flat c*WP+x, x<W
LMT = 3 * FDS      # lmh row stride
ST = 4 * FDS       # S row stride
TCT = 2 * FDC      # TC row stride
NS1 = 2 * LMT      # full s1 flat span (9252)


def _legalize_waits(nc, mybir):
    """Hoist excess sync-waits into a preceding same-engine EventSemaphore.
    The TRN2 ISA allows 1 sync-wait on compute instructions (EventSemaphore
    allows more) but Tile's scheduler can emit more; a wait-only instruction
    earlier in the same engine's program order is semantically identical."""
    limits = {"InstEventSemaphore": 2}
    n_hoisted = 0
    for f in nc.m.functions:
        for bb in f.blocks:
            il = bb.instructions
            idx = 0
            while idx < len(il):
                i = il[idx]
                si = i.sync_info
                lim = limits.get(type(i).__name__, 1)
                if si is not None and si.on_wait and len(si.on_wait) > lim:
                    waits = list(si.on_wait)
                    keep, excess = waits[:lim], waits[lim:]
                    hoists = []
                    for j in range(0, len(excess), 2):
                        h = mybir.InstEventSemaphore(
                            name=f"hoistw_{n_hoisted}", ins=[], outs=[])
                        n_hoisted += 1
                        h.engine = i.engine
                        h.sync_info = mybir.SyncInfo(
                            on_wait=excess[j:j + 2], on_update=[])
                        hoists.append(h)
                    i.sync_info = mybir.SyncInfo(
                        on_wait=keep, on_update=si.on_update)
                    for k, h in enumerate(hoists):
                        il.insert(idx + k, h)
                    idx += len(hoists)
                idx += 1
    return n_hoisted


def _build_nc():
    from concourse import bass
    import concourse.mybir as mybir
    from concourse.tile import TileContext

    bf16 = mybir.dt.bfloat16
    MIN = mybir.AluOpType.min
    MAX = mybir.AluOpType.max
    AP = bass.AP

    nc = bass.Bass()
    img = nc.dram_tensor("image", [BPC, HP, C, WP], bf16, kind="ExternalInput")
    out = nc.dram_tensor("out", [BPC, H, C, W], bf16, kind="ExternalOutput")

    def fv(tile_ap, off, dims):
        """Free-dim view of an SBUF tile: keep partition dim, replace free
        dims with `dims`, shift base by `off` elements."""
        return AP(tile_ap.tensor, tile_ap.offset + off,
                  [list(tile_ap.ap[0])] + [list(d) for d in dims])

    def stage_a(nc, pool, g, h, split_dma):
        """DMA 4 window rows/partition + shared-pair vertical sort3 +
        ScalarE shift copy. Partition p owns output rows MH*h+2p, +2p+1;
        in4[k] = padded row MH*h + 2p + k.

        Returns (lmh, s1), both [RT, 2(row), 3(slice), C, WP].
        """
        base = g * SB + h * MH * SRP
        in4 = pool.tile([RT, 4, C, WP], bf16, tag="in4")
        i4 = in4[:]
        if split_dma:
            # middle rows first: the shared pair needs only in4[1..2]
            nc.sync.dma_start(
                out=fv(i4, FDS, [[1, 2 * FDS]]),
                in_=AP(img, base + SRP, [[2 * SRP, RT], [1, 2 * FDS]]))
            nc.sync.dma_start(
                out=fv(i4, 0, [[1, FDS]]),
                in_=AP(img, base, [[2 * SRP, RT], [1, FDS]]))
            nc.sync.dma_start(
                out=fv(i4, 3 * FDS, [[1, FDS]]),
                in_=AP(img, base + 3 * SRP, [[2 * SRP, RT], [1, FDS]]))
        else:
            nc.sync.dma_start(
                out=fv(i4, 0, [[1, 4 * FDS]]),
                in_=AP(img, base, [[2 * SRP, RT], [1, 4 * FDS]]))

        TT = nc.vector.tensor_tensor
        # shared vertical pair of the strip: PM[0]=min, PM[1]=max of rows 1,2
        PM = pool.tile([RT, 2, C, WP], bf16, tag="PM", bufs=1)
        pmn = fv(PM[:], 0, [[1, FDS]])
        pmx = fv(PM[:], FDS, [[1, FDS]])
        TT(pmn, fv(i4, FDS, [[1, FDS]]), fv(i4, 2 * FDS, [[1, FDS]]), MIN)
        TT(pmx, fv(i4, FDS, [[1, FDS]]), fv(i4, 2 * FDS, [[1, FDS]]), MAX)

        # per-row combine with the third row (in4[0] for row0, in4[3] for
        # row1): m=min(pmx,3rd) hi=max(pmx,3rd) md=max(pmn,m) lo=min(pmn,m)
        lmh = pool.tile([RT, 2, 3, C, WP], bf16, tag="lmh")
        m = pool.tile([RT, C, WP], bf16, tag="m", bufs=1)
        lv, mm = lmh[:], m[:]
        for r, third_off in ((0, 0), (1, 3 * FDS)):
            third = fv(i4, third_off, [[1, FDS]])
            lo = fv(lv, r * LMT + 0 * FDS, [[1, FDS]])
            md = fv(lv, r * LMT + 1 * FDS, [[1, FDS]])
            hi = fv(lv, r * LMT + 2 * FDS, [[1, FDS]])
            TT(mm, pmx, third, MIN)
            TT(hi, pmx, third, MAX)
            TT(md, pmn, mm, MAX)
            TT(lo, pmn, mm, MIN)

        # the single odd shift, on ScalarE: s1[r][k][x] = lmh[r][k][x+1]
        s1 = pool.tile([RT, 2, 3, C, WP], bf16, tag="s1")
        nc.scalar.copy(fv(s1[:], 0, [[1, NS1 - 1]]),
                       fv(lv, 1, [[1, NS1 - 1]]))
        # last element feeds only a discarded pad column; init to keep
        # uninitialized-read checks quiet (GpSimd so VectorE loses no slot)
        nc.gpsimd.memset(fv(s1[:], NS1 - 1, [[1, 1]]), 0.0)
        return lmh, s1

    def stage_b(nc, pool, g, h, lmh, s1, split_tail):
        """Pairs + combine + med3 + DMA out, consuming stage_a tiles."""
        lv, sv = lmh[:], s1[:]
        TT = nc.vector.tensor_tensor
        # S: 0=Pmax_lo 1=Pmax_md 2=Pmin_md 3=Pmin_hi (per row)
        S = pool.tile([RT, 2, 4, C, WP], bf16, tag="S", bufs=1)
        Sv = S[:]
        d2 = [[LMT, 2], [FDS, 2], [1, FDS]]
        TT(fv(Sv, 0, [[ST, 2], [FDS, 2], [1, FDS]]),
           fv(lv, 0, d2), fv(sv, 0, d2), MAX)
        TT(fv(Sv, 2 * FDS, [[ST, 2], [FDS, 2], [1, FDS]]),
           fv(lv, FDS, d2), fv(sv, FDS, d2), MIN)

        # combine with even +2 shifts:
        # TC[r][0]=t3=min(Pmax_md, md+2); TC[r][1]=Cc=min(Pmin_hi, hi+2)
        TC = pool.tile([RT, 2, 2, FDC], bf16, tag="TC", bufs=1)
        A = pool.tile([RT, 2, FDC], bf16, tag="A", bufs=1)
        Bt = pool.tile([RT, 2, FDC], bf16, tag="Bt", bufs=1)
        TT(fv(TC[:], 0, [[TCT, 2], [FDC, 2], [1, FDC]]),
           fv(Sv, FDS, [[ST, 2], [2 * FDS, 2], [1, FDC]]),
           fv(lv, FDS + 2, [[LMT, 2], [FDS, 2], [1, FDC]]),
           MIN)
        TT(fv(A[:], 0, [[FDC, 2], [1, FDC]]),
           fv(Sv, 0, [[ST, 2], [1, FDC]]),
           fv(lv, 2, [[LMT, 2], [1, FDC]]),
           MAX)
        TT(fv(Bt[:], 0, [[FDC, 2], [1, FDC]]),
           fv(Sv, 2 * FDS, [[ST, 2], [1, FDC]]),
           fv(TC[:], 0, [[TCT, 2], [1, FDC]]),
           MAX)

        # final med3(A, B, Cc) (A in-place), then store per row parity:
        # out row MH*h + 2p + r lives at res[r]; out[.., c, x] = res[r][c*WP+x]
        f1 = pool.tile([RT, 2, FDC], bf16, tag="f1", bufs=1)
        res = pool.tile([RT, 2, FDC], bf16, tag="res")
        parities = ((0, 1), (1, 1)) if split_tail else ((0, 2),)
        for r0, nr in parities:
            dc = [[FDC, nr], [1, FDC]]
            off = r0 * FDC
            Av = fv(A[:], off, dc)
            Bv = fv(Bt[:], off, dc)
            Fv = fv(f1[:], off, dc)
            Cv = fv(TC[:], r0 * TCT + FDC, [[TCT, nr], [1, FDC]])
            TT(Fv, Av, Bv, MIN)
            TT(Av, Av, Bv, MAX)
            TT(Av, Av, Cv, MIN)
            TT(fv(res[:], off, dc), Fv, Av, MAX)
            for r in range(r0, r0 + nr):
                nc.sync.dma_start(
                    out=AP(out, g * SBO + (h * MH + r) * SR,
                           [[2 * SR, RT], [W, C], [1, W]]),
                    in_=fv(res[:], r * FDC, [[WP, C], [1, W]]))

    with TileContext(nc) as tc:
        with tc.tile_pool(name="p", bufs=2) as pool:
            # Software pipeline: emit stage_a(t+1) before stage_b(t) so the
            # ScalarE shift-copy of megastep t overlaps VectorE's vertical
            # sort of megastep t+1 instead of stalling the pair stage.
            steps = [(g, h) for g in range(BPC) for h in range(NMS)]
            prev = None
            for i, (g, h) in enumerate(steps):
                cur = stage_a(nc, pool, g, h, split_dma=(i == 0))
                if prev is not None:
                    stage_b(nc, pool, *prev[0], *prev[1], split_tail=False)
                prev = ((g, h), cur)
            stage_b(nc, pool, *prev[0], *prev[1], split_tail=True)

    _legalize_waits(nc, mybir)
    return nc


def _stage_input(img_k: np.ndarray) -> np.ndarray:
    """[BPC, C, H, W] f32 -> reflect-padded transposed [BPC, HP, C, WP] bf16."""
    import ml_dtypes

    t = img_k.transpose(0, 2, 1, 3).astype(ml_dtypes.bfloat16)  # [BPC,H,C,W]
    p = np.empty((BPC, HP, C, WP), dtype=ml_dtypes.bfloat16)
    p[:, 1:H + 1, :, 1:W + 1] = t
    p[:, 1:H + 1, :, 0] = t[:, :, :, 1]          # col -1 = col 1
    p[:, 1:H + 1, :, W + 1] = t[:, :, :, W - 2]  # col W  = col W-2
    p[:, 0] = p[:, 2]          # row -1 = row 1
    p[:, H + 1] = p[:, H - 1]  # row H  = row H-2
    return p


def kernel(image: np.ndarray) -> np.ndarray:
    from concourse.bass_utils import run_bass_kernel_spmd

    image = np.asarray(image, dtype=np.float32)
    if "nc" not in _COMPILED:
        _COMPILED["nc"] = _build_nc()
    nc = _COMPILED["nc"]

    in_maps = [{"image": _stage_input(image[k * BPC:(k + 1) * BPC])}
               for k in range(NCORES)]
    try:
        res = run_bass_kernel_spmd(nc, in_maps, core_ids=list(range(NCORES)))
    except Exception:
        # transient accelerator errors (e.g. NRT_EXEC_UNIT_UNRECOVERABLE)
        # have been observed to clear on retry
        res = run_bass_kernel_spmd(nc, in_maps, core_ids=list(range(NCORES)))
    return np.concatenate(
        [np.asarray(res.results[k]["out"]).astype(np.float32)
         .transpose(0, 2, 1, 3) for k in range(NCORES)],
        axis=0)


# revision 18
# speedup vs baseline: 1.0133x; 1.0066x over previous
"""3x3 median filter (reflect padding) on Trainium2, data-parallel over batch.

Input:  image [16, 3, 512, 512] f32
Output: same shape; out[b,c,y,x] = median of the 3x3 window around (y,x),
        reflect padding.

Sharding: batch dim split across 8 NeuronCores (2 images per core), SPMD.

All VectorE TENSOR_TENSOR ops run in bf16 with dense step-1, 4-byte-aligned
access patterns so every one hits the DVE 2x_1P perf mode (2 elem/cycle/lane)
instead of the 1x floor fp32 TT is stuck at. bf16 keeps rel-err ~2^-8
(<< 2e-2 tolerance) with a full 8-bit exponent (no subnormal blowup near
the harness' 1e-6 denom floor).

Host prep: per-core input is transposed+padded to [BPC, H+2, C, W+2] bf16
with BOTH the vertical and horizontal reflect borders pre-staged. The
horizontal pad removes all boundary-column special cases; every op is a
uniform dense sweep over flattened [C, W+2] (the 2 pad cols per channel
compute garbage that is simply not stored).

Layout: 2-row strips. Each megastep covers a 256-row half-image; partition
p owns output rows 2p and 2p+1 and loads its 4 window rows (2p..2p+3
padded) with ONE contiguous DMA (1.5x HBM read amplification vs 3x for
row-per-partition layouts). The two output rows share their middle
vertical pair, cutting the vertical sort from 6 to 5 min/max ops per row:
  PM = min/max(row1, row2)                     (2 TT, shared by both rows)
  per row: m, hi, md, lo from PM + third row   (8 TT)
Horizontal merge per megastep (ScalarE makes the single odd-shift copy
s1[k][x]=lmh[k][x+1]; everything else is even offsets):
  pairs:   Pmax_lo,Pmax_md / Pmin_md,Pmin_hi   (2 stacked TT)
  combine: t3, Cc (stacked) / A / B            (3 TT)
  median = med3(A, B, Cc)                      (4 TT)
Software pipeline: stage_a(t+1) [DMA+vertical+s1] is emitted before
stage_b(t) [pairs..median] so the ScalarE shift-copy never stalls VectorE.
The first megastep's DMA is split so compute starts after the 2 middle
rows land; the last megastep's median+store are split by row parity so
the final DMA drains only half the result.

Measured: ~133 us HW exec for the full [16,3,512,512] input across 8
cores (vs 236 us for the all-fp32 row-per-partition version), rel err
3.9e-3 (pure bf16 rounding). VectorE ~121.5 us busy and gap-free after
warmup -- 17 TT ops/pixel at 2 elem/cycle/lane is the engine floor for
this decomposition; head (~12 us NEFF preamble + first DMA) and tail
(~5 us final store) account for the rest.
"""

import sys

sys.path.insert(0, "/opt/trn_rl_repo")

import numpy as np

_COMPILED = {}

B, C, H, W = 16, 3, 512, 512
NCORES = 8
BPC = B // NCORES  # batches per core
RT = 128           # partitions
NMS = 2            # megasteps (half-images) per batch
MH = H // NMS      # rows per megastep (256)
HP = H + 2         # padded rows on device
WP = W + 2         # padded cols on device
SRP = C * WP       # padded row stride (elements), 1542
SB = HP * SRP      # batch stride (input)
SR = C * W         # output row stride
SBO = H * SR       # batch stride (output)
FDS = SRP          # one row-slice, flattened [C, WP]
FDC = 2 * WP + W   # combine width 1540: covers flat c*WP+x, x<W
LMT = 3 * FDS      # lmh row stride
ST = 4 * FDS       # S row stride
TCT = 2 * FDC      # TC row stride
NS1 = 2 * LMT      # full s1 flat span (9252)


def _legalize_waits(nc, mybir):
    """Hoist excess sync-waits into a preceding same-engine EventSemaphore.
    The TRN2 ISA allows 1 sync-wait on compute instructions (EventSemaphore
    allows more) but Tile's scheduler can emit more; a wait-only instruction
    earlier in the same engine's program order is semantically identical."""
    limits = {"InstEventSemaphore": 2}
    n_hoisted = 0
    for f in nc.m.functions:
        for bb in f.blocks:
            il = bb.instructions
            idx = 0
            while idx < len(il):
                i = il[idx]
                si = i.sync_info
                lim = limits.get(type(i).__name__, 1)
                if si is not None and si.on_wait and len(si.on_wait) > lim:
                    waits = list(si.on_wait)
                    keep, excess = waits[:lim], waits[lim:]
                    hoists = []
                    for j in range(0, len(excess), 2):
                        h = mybir.InstEventSemaphore(
                            name=f"hoistw_{n_hoisted}", ins=[], outs=[])
                        n_hoisted += 1
                        h.engine = i.engine
                        h.sync_info = mybir.SyncInfo(
                            on_wait=excess[j:j + 2], on_update=[])
                        hoists.append(h)
                    i.sync_info = mybir.SyncInfo(
                        on_wait=keep, on_update=si.on_update)
                    for k, h in enumerate(hoists):
                        il.insert(idx + k, h)
                    idx += len(hoists)
                idx += 1
    return n_hoisted


def _build_nc():
    from concourse import bass
    import concourse.mybir as mybir
    from concourse.tile import TileContext

    bf16 = mybir.dt.bfloat16
    MIN = mybir.AluOpType.min
    MAX = mybir.AluOpType.max
    AP = bass.AP

    nc = bass.Bass()
    img = nc.dram_tensor("image", [BPC, HP, C, WP], bf16, kind="ExternalInput")
    out = nc.dram_tensor("out", [BPC, H, C, W], bf16, kind="ExternalOutput")

    def fv(tile_ap, off, dims):
        """Free-dim view of an SBUF tile: keep partition dim, replace free
        dims with `dims`, shift base by `off` elements."""
        return AP(tile_ap.tensor, tile_ap.offset + off,
                  [list(tile_ap.ap[0])] + [list(d) for d in dims])

    def stage_a(nc, pool, g, h, split_dma):
        """DMA 4 window rows/partition + shared-pair vertical sort3 +
        ScalarE shift copy. Partition p owns output rows MH*h+2p, +2p+1;
        in4[k] = padded row MH*h + 2p + k.

        Returns (lmh, s1), both [RT, 2(row), 3(slice), C, WP].
        """
        base = g * SB + h * MH * SRP
        in4 = pool.tile([RT, 4, C, WP], bf16, tag="in4")
        i4 = in4[:]
        if split_dma:
            # middle rows first: the shared pair needs only in4[1..2]
            nc.sync.dma_start(
                out=fv(i4, FDS, [[1, 2 * FDS]]),
                in_=AP(img, base + SRP, [[2 * SRP, RT], [1, 2 * FDS]]))
            nc.sync.dma_start(
                out=fv(i4, 0, [[1, FDS]]),
                in_=AP(img, base, [[2 * SRP, RT], [1, FDS]]))
            nc.sync.dma_start(
                out=fv(i4, 3 * FDS, [[1, FDS]]),
                in_=AP(img, base + 3 * SRP, [[2 * SRP, RT], [1, FDS]]))
        else:
            nc.sync.dma_start(
                out=fv(i4, 0, [[1, 4 * FDS]]),
                in_=AP(img, base, [[2 * SRP, RT], [1, 4 * FDS]]))

        TT = nc.vector.tensor_tensor
        # shared vertical pair of the strip: PM[0]=min, PM[1]=max of rows 1,2
        PM = pool.tile([RT, 2, C, WP], bf16, tag="PM", bufs=1)
        pmn = fv(PM[:], 0, [[1, FDS]])
        pmx = fv(PM[:], FDS, [[1, FDS]])
        TT(pmn, fv(i4, FDS, [[1, FDS]]), fv(i4, 2 * FDS, [[1, FDS]]), MIN)
        TT(pmx, fv(i4, FDS, [[1, FDS]]), fv(i4, 2 * FDS, [[1, FDS]]), MAX)

        # per-row combine with the third row (in4[0] for row0, in4[3] for
        # row1): m=min(pmx,3rd) hi=max(pmx,3rd) md=max(pmn,m) lo=min(pmn,m)
        lmh = pool.tile([RT, 2, 3, C, WP], bf16, tag="lmh")
        m = pool.tile([RT, C, WP], bf16, tag="m", bufs=1)
        lv, mm = lmh[:], m[:]
        for r, third_off in ((0, 0), (1, 3 * FDS)):
            third = fv(i4, third_off, [[1, FDS]])
            lo = fv(lv, r * LMT + 0 * FDS, [[1, FDS]])
            md = fv(lv, r * LMT + 1 * FDS, [[1, FDS]])
            hi = fv(lv, r * LMT + 2 * FDS, [[1, FDS]])
            TT(mm, pmx, third, MIN)
            TT(hi, pmx, third, MAX)
            TT(md, pmn, mm, MAX)
            TT(lo, pmn, mm, MIN)

        # the single odd shift, on ScalarE: s1[r][k][x] = lmh[r][k][x+1]
        s1 = pool.tile([RT, 2, 3, C, WP], bf16, tag="s1")
        nc.scalar.copy(fv(s1[:], 0, [[1, NS1 - 1]]),
                       fv(lv, 1, [[1, NS1 - 1]]))
        # last element feeds only a discarded pad column; init to keep
        # uninitialized-read checks quiet
        nc.vector.memset(fv(s1[:], NS1 - 1, [[1, 1]]), 0.0)
        return lmh, s1

    def stage_b(nc, pool, g, h, lmh, s1, split_tail):
        """Pairs + combine + med3 + DMA out, consuming stage_a tiles."""
        lv, sv = lmh[:], s1[:]
        TT = nc.vector.tensor_tensor
        # S: 0=Pmax_lo 1=Pmax_md 2=Pmin_md 3=Pmin_hi (per row)
        S = pool.tile([RT, 2, 4, C, WP], bf16, tag="S", bufs=1)
        Sv = S[:]
        d2 = [[LMT, 2], [FDS, 2], [1, FDS]]
        TT(fv(Sv, 0, [[ST, 2], [FDS, 2], [1, FDS]]),
           fv(lv, 0, d2), fv(sv, 0, d2), MAX)
        TT(fv(Sv, 2 * FDS, [[ST, 2], [FDS, 2], [1, FDS]]),
           fv(lv, FDS, d2), fv(sv, FDS, d2), MIN)

        # combine with even +2 shifts:
        # TC[r][0]=t3=min(Pmax_md, md+2); TC[r][1]=Cc=min(Pmin_hi, hi+2)
        TC = pool.tile([RT, 2, 2, FDC], bf16, tag="TC", bufs=1)
        A = pool.tile([RT, 2, FDC], bf16, tag="A", bufs=1)
        Bt = pool.tile([RT, 2, FDC], bf16, tag="Bt", bufs=1)
        TT(fv(TC[:], 0, [[TCT, 2], [FDC, 2], [1, FDC]]),
           fv(Sv, FDS, [[ST, 2], [2 * FDS, 2], [1, FDC]]),
           fv(lv, FDS + 2, [[LMT, 2], [FDS, 2], [1, FDC]]),
           MIN)
        TT(fv(A[:], 0, [[FDC, 2], [1, FDC]]),
           fv(Sv, 0, [[ST, 2], [1, FDC]]),
           fv(lv, 2, [[LMT, 2], [1, FDC]]),
           MAX)
        TT(fv(Bt[:], 0, [[FDC, 2], [1, FDC]]),
           fv(Sv, 2 * FDS, [[ST, 2], [1, FDC]]),
           fv(TC[:], 0, [[TCT, 2], [1, FDC]]),
           MAX)

        # final med3(A, B, Cc) (A in-place), then store per row parity:
        # out row MH*h + 2p + r lives at res[r]; out[.., c, x] = res[r][c*WP+x]
        f1 = pool.tile([RT, 2, FDC], bf16, tag="f1", bufs=1)
        res = pool.tile([RT, 2, FDC], bf16, tag="res")
        parities = ((0, 1), (1, 1)) if split_tail else ((0, 2),)
        for r0, nr in parities:
            dc = [[FDC, nr], [1, FDC]]
            off = r0 * FDC
            Av = fv(A[:], off, dc)
            Bv = fv(Bt[:], off, dc)
            Fv = fv(f1[:], off, dc)
            Cv = fv(TC[:], r0 * TCT + FDC, [[TCT, nr], [1, FDC]])
            TT(Fv, Av, Bv, MIN)
            TT(Av, Av, Bv, MAX)
            TT(Av, Av, Cv, MIN)
            TT(fv(res[:], off, dc), Fv, Av, MAX)
            for r in range(r0, r0 + nr):
                nc.sync.dma_start(
                    out=AP(out, g * SBO + (h * MH + r) * SR,
                           [[2 * SR, RT], [W, C], [1, W]]),
                    in_=fv(res[:], r * FDC, [[WP, C], [1, W]]))

    with TileContext(nc) as tc:
        with tc.tile_pool(name="p", bufs=2) as pool:
            # Software pipeline: emit stage_a(t+1) before stage_b(t) so the
            # ScalarE shift-copy of megastep t overlaps VectorE's vertical
            # sort of megastep t+1 instead of stalling the pair stage.
            steps = [(g, h) for g in range(BPC) for h in range(NMS)]
            prev = None
            for i, (g, h) in enumerate(steps):
                cur = stage_a(nc, pool, g, h, split_dma=(i == 0))
                if prev is not None:
                    stage_b(nc, pool, *prev[0], *prev[1], split_tail=False)
                prev = ((g, h), cur)
            stage_b(nc, pool, *prev[0], *prev[1], split_tail=True)

    _legalize_waits(nc, mybir)
    return nc


def _stage_input(img_k: np.ndarray) -> np.ndarray:
    """[BPC, C, H, W] f32 -> reflect-padded transposed [BPC, HP, C, WP] bf16."""
    import ml_dtypes

    t = img_k.transpose(0, 2, 1, 3).astype(ml_dtypes.bfloat16)  # [BPC,H,C,W]
    p = np.empty((BPC, HP, C, WP), dtype=ml_dtypes.bfloat16)
    p[:, 1:H + 1, :, 1:W + 1] = t
    p[:, 1:H + 1, :, 0] = t[:, :, :, 1]          # col -1 = col 1
    p[:, 1:H + 1, :, W + 1] = t[:, :, :, W - 2]  # col W  = col W-2
    p[:, 0] = p[:, 2]          # row -1 = row 1
    p[:, H + 1] = p[:, H - 1]  # row H  = row H-2
    return p


def kernel(image: np.ndarray) -> np.ndarray:
    from concourse.bass_utils import run_bass_kernel_spmd

    image = np.asarray(image, dtype=np.float32)
    if "nc" not in _COMPILED:
        _COMPILED["nc"] = _build_nc()
    nc = _COMPILED["nc"]

    in_maps = [{"image": _stage_input(image[k * BPC:(k + 1) * BPC])}
               for k in range(NCORES)]
    try:
        res = run_bass_kernel_spmd(nc, in_maps, core_ids=list(range(NCORES)))
    except Exception:
        # transient accelerator errors (e.g. NRT_EXEC_UNIT_UNRECOVERABLE)
        # have been observed to clear on retry
        res = run_bass_kernel_spmd(nc, in_maps, core_ids=list(range(NCORES)))
    return np.concatenate(
        [np.asarray(res.results[k]["out"]).astype(np.float32)
         .transpose(0, 2, 1, 3) for k in range(NCORES)],
        axis=0)


# revision 19
# speedup vs baseline: 1.0211x; 1.0078x over previous
"""3x3 median filter (reflect padding) on Trainium2, data-parallel over batch.

Input:  image [16, 3, 512, 512] f32
Output: same shape; out[b,c,y,x] = median of the 3x3 window around (y,x),
        reflect padding.

Sharding: batch dim split across 8 NeuronCores (2 images per core), SPMD.

All VectorE TENSOR_TENSOR ops run in bf16 with dense step-1, 4-byte-aligned
access patterns so every one hits the DVE 2x_1P perf mode (2 elem/cycle/lane)
instead of the 1x floor fp32 TT is stuck at. bf16 keeps rel-err ~2^-8
(<< 2e-2 tolerance) with a full 8-bit exponent (no subnormal blowup near
the harness' 1e-6 denom floor).

Host prep: per-core input is transposed+padded to [BPC, H+2, C, W+2] bf16
with BOTH the vertical and horizontal reflect borders pre-staged. The
horizontal pad removes all boundary-column special cases; every op is a
uniform dense sweep over flattened [C, W+2] (the 2 pad cols per channel
compute garbage that is simply not stored).

Layout: 2-row strips. Each megastep covers a 256-row half-image; partition
p owns output rows 2p and 2p+1 and loads its 4 window rows (2p..2p+3
padded) with ONE contiguous DMA (1.5x HBM read amplification vs 3x for
row-per-partition layouts). The two output rows share their middle
vertical pair, cutting the vertical sort from 6 to 5 min/max ops per row:
  PM = min/max(row1, row2)                     (2 TT, shared by both rows)
  per row: m, hi, md, lo from PM + third row   (8 TT)
Horizontal merge per megastep (ScalarE makes the single odd-shift copy
s1[k][x]=lmh[k][x+1]; everything else is even offsets):
  pairs:   Pmax_lo,Pmax_md / Pmin_md,Pmin_hi   (2 stacked TT)
  combine: t3, Cc (stacked) / A / B            (3 TT)
  median = med3(A, B, Cc)                      (4 TT)
Software pipeline: stage_a(t+1) [DMA+vertical+s1] is emitted before
stage_b(t) [pairs..median] so the ScalarE shift-copy never stalls VectorE.
The first megastep's DMA is split so compute starts after the 2 middle
rows land; the last megastep's median+store are split by row parity so
the final DMA drains only half the result.

Measured: ~133 us HW exec for the full [16,3,512,512] input across 8
cores (vs 236 us for the all-fp32 row-per-partition version), rel err
3.9e-3 (pure bf16 rounding). VectorE ~121.5 us busy and gap-free after
warmup -- 17 TT ops/pixel at 2 elem/cycle/lane is the engine floor for
this decomposition; head (~12 us NEFF preamble + first DMA) and tail
(~5 us final store) account for the rest.
"""

import sys

sys.path.insert(0, "/opt/trn_rl_repo")

import numpy as np

_COMPILED = {}

B, C, H, W = 16, 3, 512, 512
NCORES = 8
BPC = B // NCORES  # batches per core
RT = 128           # partitions
NMS = 2            # megasteps (half-images) per batch
MH = H // NMS      # rows per megastep (256)
HP = H + 2         # padded rows on device
WP = W + 2         # padded cols on device
SRP = C * WP       # padded row stride (elements), 1542
SB = HP * SRP      # batch stride (input)
SR = C * W         # output row stride
SBO = H * SR       # batch stride (output)
FDS = SRP          # one row-slice, flattened [C, WP]
FDC = 2 * WP + W   # combine width 1540: covers flat c*WP+x, x<W
LMT = 3 * FDS      # lmh row stride
ST = 4 * FDS       # S row stride
TCT = 2 * FDC      # TC row stride
NS1 = 2 * LMT      # full s1 flat span (9252)


def _legalize_waits(nc, mybir):
    """Hoist excess sync-waits into a preceding same-engine EventSemaphore.
    The TRN2 ISA allows 1 sync-wait on compute instructions (EventSemaphore
    allows more) but Tile's scheduler can emit more; a wait-only instruction
    earlier in the same engine's program order is semantically identical."""
    limits = {"InstEventSemaphore": 2}
    n_hoisted = 0
    for f in nc.m.functions:
        for bb in f.blocks:
            il = bb.instructions
            idx = 0
            while idx < len(il):
                i = il[idx]
                si = i.sync_info
                lim = limits.get(type(i).__name__, 1)
                if si is not None and si.on_wait and len(si.on_wait) > lim:
                    waits = list(si.on_wait)
                    keep, excess = waits[:lim], waits[lim:]
                    hoists = []
                    for j in range(0, len(excess), 2):
                        h = mybir.InstEventSemaphore(
                            name=f"hoistw_{n_hoisted}", ins=[], outs=[])
                        n_hoisted += 1
                        h.engine = i.engine
                        h.sync_info = mybir.SyncInfo(
                            on_wait=excess[j:j + 2], on_update=[])
                        hoists.append(h)
                    i.sync_info = mybir.SyncInfo(
                        on_wait=keep, on_update=si.on_update)
                    for k, h in enumerate(hoists):
                        il.insert(idx + k, h)
                    idx += len(hoists)
                idx += 1
    return n_hoisted


def _build_nc():
    from concourse import bass
    import concourse.mybir as mybir
    from concourse.tile import TileContext

    bf16 = mybir.dt.bfloat16
    MIN = mybir.AluOpType.min
    MAX = mybir.AluOpType.max
    AP = bass.AP

    nc = bass.Bass()
    img = nc.dram_tensor("image", [BPC, HP, C, WP], bf16, kind="ExternalInput")
    out = nc.dram_tensor("out", [BPC, H, C, W], bf16, kind="ExternalOutput")

    def fv(tile_ap, off, dims):
        """Free-dim view of an SBUF tile: keep partition dim, replace free
        dims with `dims`, shift base by `off` elements."""
        return AP(tile_ap.tensor, tile_ap.offset + off,
                  [list(tile_ap.ap[0])] + [list(d) for d in dims])

    def stage_a(nc, pool, g, h, split_dma):
        """DMA 4 window rows/partition + shared-pair vertical sort3 +
        ScalarE shift copy. Partition p owns output rows MH*h+2p, +2p+1;
        in4[k] = padded row MH*h + 2p + k.

        Returns (lmh, s1), both [RT, 2(row), 3(slice), C, WP].
        """
        base = g * SB + h * MH * SRP
        in4 = pool.tile([RT, 4, C, WP], bf16, tag="in4")
        i4 = in4[:]
        if split_dma:
            # middle rows first: the shared pair needs only in4[1..2]
            nc.sync.dma_start(
                out=fv(i4, FDS, [[1, 2 * FDS]]),
                in_=AP(img, base + SRP, [[2 * SRP, RT], [1, 2 * FDS]]))
            nc.sync.dma_start(
                out=fv(i4, 0, [[1, FDS]]),
                in_=AP(img, base, [[2 * SRP, RT], [1, FDS]]))
            nc.sync.dma_start(
                out=fv(i4, 3 * FDS, [[1, FDS]]),
                in_=AP(img, base + 3 * SRP, [[2 * SRP, RT], [1, FDS]]))
        else:
            nc.sync.dma_start(
                out=fv(i4, 0, [[1, 4 * FDS]]),
                in_=AP(img, base, [[2 * SRP, RT], [1, 4 * FDS]]))

        TT = nc.vector.tensor_tensor
        # shared vertical pair of the strip: PM[0]=min, PM[1]=max of rows 1,2
        PM = pool.tile([RT, 2, C, WP], bf16, tag="PM", bufs=1)
        pmn = fv(PM[:], 0, [[1, FDS]])
        pmx = fv(PM[:], FDS, [[1, FDS]])
        TT(pmn, fv(i4, FDS, [[1, FDS]]), fv(i4, 2 * FDS, [[1, FDS]]), MIN)
        TT(pmx, fv(i4, FDS, [[1, FDS]]), fv(i4, 2 * FDS, [[1, FDS]]), MAX)

        # combine with the third row (in4[0] for row0, in4[3] for row1),
        # both rows stacked per instruction; the shared PM slices enter via
        # 0-stride broadcast dims (verified to keep the 2x_1P perf mode):
        # m=min(pmx,3rd) hi=max(pmx,3rd) md=max(pmn,m) lo=min(pmn,m)
        lmh = pool.tile([RT, 2, 3, C, WP], bf16, tag="lmh")
        m = pool.tile([RT, 2, C, WP], bf16, tag="m", bufs=1)
        lv = lmh[:]
        thirds = fv(i4, 0, [[3 * FDS, 2], [1, FDS]])
        pmn_b = fv(PM[:], 0, [[0, 2], [1, FDS]])
        pmx_b = fv(PM[:], FDS, [[0, 2], [1, FDS]])
        mm = fv(m[:], 0, [[FDS, 2], [1, FDS]])
        lo = fv(lv, 0 * FDS, [[LMT, 2], [1, FDS]])
        md = fv(lv, 1 * FDS, [[LMT, 2], [1, FDS]])
        hi = fv(lv, 2 * FDS, [[LMT, 2], [1, FDS]])
        TT(mm, pmx_b, thirds, MIN)
        TT(hi, pmx_b, thirds, MAX)
        TT(md, pmn_b, mm, MAX)
        TT(lo, pmn_b, mm, MIN)

        # the single odd shift, on ScalarE: s1[r][k][x] = lmh[r][k][x+1]
        s1 = pool.tile([RT, 2, 3, C, WP], bf16, tag="s1")
        nc.scalar.copy(fv(s1[:], 0, [[1, NS1 - 1]]),
                       fv(lv, 1, [[1, NS1 - 1]]))
        # last element feeds only a discarded pad column; init to keep
        # uninitialized-read checks quiet
        nc.vector.memset(fv(s1[:], NS1 - 1, [[1, 1]]), 0.0)
        return lmh, s1

    def stage_b(nc, pool, g, h, lmh, s1, split_tail):
        """Pairs + combine + med3 + DMA out, consuming stage_a tiles."""
        lv, sv = lmh[:], s1[:]
        TT = nc.vector.tensor_tensor
        # S: 0=Pmax_lo 1=Pmax_md 2=Pmin_md 3=Pmin_hi (per row)
        S = pool.tile([RT, 2, 4, C, WP], bf16, tag="S", bufs=1)
        Sv = S[:]
        d2 = [[LMT, 2], [FDS, 2], [1, FDS]]
        TT(fv(Sv, 0, [[ST, 2], [FDS, 2], [1, FDS]]),
           fv(lv, 0, d2), fv(sv, 0, d2), MAX)
        TT(fv(Sv, 2 * FDS, [[ST, 2], [FDS, 2], [1, FDS]]),
           fv(lv, FDS, d2), fv(sv, FDS, d2), MIN)

        # combine with even +2 shifts:
        # TC[r][0]=t3=min(Pmax_md, md+2); TC[r][1]=Cc=min(Pmin_hi, hi+2)
        TC = pool.tile([RT, 2, 2, FDC], bf16, tag="TC", bufs=1)
        A = pool.tile([RT, 2, FDC], bf16, tag="A", bufs=1)
        Bt = pool.tile([RT, 2, FDC], bf16, tag="Bt", bufs=1)
        TT(fv(TC[:], 0, [[TCT, 2], [FDC, 2], [1, FDC]]),
           fv(Sv, FDS, [[ST, 2], [2 * FDS, 2], [1, FDC]]),
           fv(lv, FDS + 2, [[LMT, 2], [FDS, 2], [1, FDC]]),
           MIN)
        TT(fv(A[:], 0, [[FDC, 2], [1, FDC]]),
           fv(Sv, 0, [[ST, 2], [1, FDC]]),
           fv(lv, 2, [[LMT, 2], [1, FDC]]),
           MAX)
        TT(fv(Bt[:], 0, [[FDC, 2], [1, FDC]]),
           fv(Sv, 2 * FDS, [[ST, 2], [1, FDC]]),
           fv(TC[:], 0, [[TCT, 2], [1, FDC]]),
           MAX)

        # final med3(A, B, Cc) (A in-place), then store per row parity:
        # out row MH*h + 2p + r lives at res[r]; out[.., c, x] = res[r][c*WP+x]
        f1 = pool.tile([RT, 2, FDC], bf16, tag="f1", bufs=1)
        res = pool.tile([RT, 2, FDC], bf16, tag="res")
        parities = ((0, 1), (1, 1)) if split_tail else ((0, 2),)
        for r0, nr in parities:
            dc = [[FDC, nr], [1, FDC]]
            off = r0 * FDC
            Av = fv(A[:], off, dc)
            Bv = fv(Bt[:], off, dc)
            Fv = fv(f1[:], off, dc)
            Cv = fv(TC[:], r0 * TCT + FDC, [[TCT, nr], [1, FDC]])
            TT(Fv, Av, Bv, MIN)
            TT(Av, Av, Bv, MAX)
            TT(Av, Av, Cv, MIN)
            TT(fv(res[:], off, dc), Fv, Av, MAX)
            for r in range(r0, r0 + nr):
                nc.sync.dma_start(
                    out=AP(out, g * SBO + (h * MH + r) * SR,
                           [[2 * SR, RT], [W, C], [1, W]]),
                    in_=fv(res[:], r * FDC, [[WP, C], [1, W]]))

    with TileContext(nc) as tc:
        with tc.tile_pool(name="p", bufs=2) as pool:
            # Software pipeline: emit stage_a(t+1) before stage_b(t) so the
            # ScalarE shift-copy of megastep t overlaps VectorE's vertical
            # sort of megastep t+1 instead of stalling the pair stage.
            steps = [(g, h) for g in range(BPC) for h in range(NMS)]
            prev = None
            for i, (g, h) in enumerate(steps):
                cur = stage_a(nc, pool, g, h, split_dma=(i == 0))
                if prev is not None:
                    stage_b(nc, pool, *prev[0], *prev[1], split_tail=False)
                prev = ((g, h), cur)
            stage_b(nc, pool, *prev[0], *prev[1], split_tail=True)

    _legalize_waits(nc, mybir)
    return nc


def _stage_input(img_k: np.ndarray) -> np.ndarray:
    """[BPC, C, H, W] f32 -> reflect-padded transposed [BPC, HP, C, WP] bf16."""
    import ml_dtypes

    t = img_k.transpose(0, 2, 1, 3).astype(ml_dtypes.bfloat16)  # [BPC,H,C,W]
    p = np.empty((BPC, HP, C, WP), dtype=ml_dtypes.bfloat16)
    p[:, 1:H + 1, :, 1:W + 1] = t
    p[:, 1:H + 1, :, 0] = t[:, :, :, 1]          # col -1 = col 1
    p[:, 1:H + 1, :, W + 1] = t[:, :, :, W - 2]  # col W  = col W-2
    p[:, 0] = p[:, 2]          # row -1 = row 1
    p[:, H + 1] = p[:, H - 1]  # row H  = row H-2
    return p


def kernel(image: np.ndarray) -> np.ndarray:
    from concourse.bass_utils import run_bass_kernel_spmd

    image = np.asarray(image, dtype=np.float32)
    if "nc" not in _COMPILED:
        _COMPILED["nc"] = _build_nc()
    nc = _COMPILED["nc"]

    in_maps = [{"image": _stage_input(image[k * BPC:(k + 1) * BPC])}
               for k in range(NCORES)]
    try:
        res = run_bass_kernel_spmd(nc, in_maps, core_ids=list(range(NCORES)))
    except Exception:
        # transient accelerator errors (e.g. NRT_EXEC_UNIT_UNRECOVERABLE)
        # have been observed to clear on retry
        res = run_bass_kernel_spmd(nc, in_maps, core_ids=list(range(NCORES)))
    return np.concatenate(
        [np.asarray(res.results[k]["out"]).astype(np.float32)
         .transpose(0, 2, 1, 3) for k in range(NCORES)],
        axis=0)
